# revision 1
# baseline (speedup 1.0000x reference)
"""CGCoupler Trainium2 Bass kernel.

out[n, ro[k]] += x1[n, r1[k]] * x2[n, r2[k]] * cg[k]  for all k, rows n.

Because the CG index tables address contiguous channel runs, the whole op
decomposes into ~147 contiguous-slice FMAs per row:
    out[:, o:o+d] += c * x1[:, a:a+d] * x2[:, b:b+d]
with d in {32, 64}.  We lay batch rows on the 128 SBUF partitions and the
640-wide feature dim on the free axis, fold T=8 row-tiles into each DVE
instruction via multi-dim access patterns, and merge slice-ops with equal
coefficient and affine offset progressions into single instructions.

Data-parallel across 8 NeuronCores: each core processes 2048 rows.
"""
import numpy as np

N_CORES = 8
P_DIM = 128
T_FOLD = 8          # row-tiles folded per DVE instruction group
N_CHUNKS = 3        # product-pair buffer chunks (SBUF sizing)

_BUILD_CACHE = {}


# ----------------------------------------------------------------------------
# Planning: decompose index tables into merged slice-op instructions
# ----------------------------------------------------------------------------

def _extract_sliceops(cg, r1, r2, ro):
    M = len(cg)
    ops = []
    k = 0
    while k < M:
        j = k + 1
        while (j < M and r1[j] == r1[j-1] + 1 and r2[j] == r2[j-1] + 1
               and ro[j] == ro[j-1] + 1 and cg[j] == cg[k]):
            j += 1
        ops.append((int(r1[k]), int(r2[k]), int(ro[k]), j - k, float(cg[k])))
        k = j
    return ops


def _build_plan(cg, r1, r2, ro, out_dim, n_chunks=N_CHUNKS):
    """Products are TensorTensor (4D APs allowed); accums are
    TensorScalarPtr (TS/STT), which the HW verifier limits to <=3D access
    patterns (partition + 2 free dims).  One free dim is the T-fold, so an
    accum instruction covers multiple slice-ops only when they collapse
    into one contiguous run (out offsets AND product slots stepping by d).
    """
    ops = _extract_sliceops(cg, r1, r2, ro)

    pair_order, pair_idx = [], {}
    for (a, b, o, d, c) in ops:
        key = (a, b, d)
        if key not in pair_idx:
            pair_idx[key] = len(pair_order)
            pair_order.append(key)

    total_elems = sum(d for (_, _, d) in pair_order)
    target = total_elems / n_chunks
    chunks, slot, chunk_sizes = [], {}, []
    cur, cur_sz = [], 0
    for key in pair_order:
        d = key[2]
        if cur_sz + d > target * 1.02 and len(chunks) < n_chunks - 1 and cur:
            chunks.append(cur); chunk_sizes.append(cur_sz)
            cur, cur_sz = [], 0
        slot[key] = (len(chunks), cur_sz)
        cur.append(key)
        cur_sz += d
    chunks.append(cur); chunk_sizes.append(cur_sz)

    # products: merge runs with constant (da, db, dslot), equal d (4D TT)
    prod_instrs = [[] for _ in range(n_chunks)]
    for ci, ch in enumerate(chunks):
        i = 0
        while i < len(ch):
            a0, b0, d0 = ch[i]
            s0 = slot[ch[i]][1]
            j = i + 1
            da = db = ds = None
            while j < len(ch):
                a1, b1, d1 = ch[j]
                if d1 != d0:
                    break
                nda = a1 - ch[j-1][0]
                ndb = b1 - ch[j-1][1]
                nds = slot[ch[j]][1] - slot[ch[j-1]][1]
                if da is None:
                    da, db, ds = nda, ndb, nds
                elif (nda, ndb, nds) != (da, db, ds):
                    break
                j += 1
            n = j - i
            if n == 1:
                da = db = ds = 0
            prod_instrs[ci].append(dict(pslot=s0, a=a0, b=b0, d=d0,
                                        da=da, db=db, ds=ds, n=n))
            i = j

    # accumulations: chunk-major, wide-first; first full-touch is a write
    acc_raw = [[] for _ in range(n_chunks)]
    for (a, b, o, d, c) in ops:
        ci, off = slot[(a, b, d)]
        acc_raw[ci].append(dict(o=o, pslot=off, c=c, d=d))
    covered = np.zeros(out_dim, bool)
    needs_memset = False
    per_chunk = []
    for ci in range(n_chunks):
        qs = sorted(acc_raw[ci], key=lambda q: (-q['d'], q['c'], q['o'], q['pslot']))
        for q in qs:
            rng = slice(q['o'], q['o'] + q['d'])
            if not covered[rng].any():
                q['kind'] = 'TS'
            else:
                if not covered[rng].all():
                    needs_memset = True
                q['kind'] = 'STT'
            covered[rng] = True
        per_chunk.append(qs)
    if not covered.all():
        needs_memset = True

    # merge only naturally-contiguous runs (collapse to [T, n*d], 3D)
    acc_instrs = [[] for _ in range(n_chunks)]
    for ci in range(n_chunks):
        qs = sorted(per_chunk[ci],
                    key=lambda q: (q['kind'] != 'TS', -q['d'], q['c'], q['o'], q['pslot']))
        i = 0
        while i < len(qs):
            q0 = qs[i]
            j = i + 1
            while j < len(qs):
                q1, qp = qs[j], qs[j-1]
                if q1['kind'] != q0['kind'] or q1['d'] != q0['d'] or q1['c'] != q0['c']:
                    break
                if q1['o'] - qp['o'] != q0['d'] or q1['pslot'] - qp['pslot'] != q0['d']:
                    break
                j += 1
            n = j - i
            acc_instrs[ci].append(dict(kind=q0['kind'], o=q0['o'], pslot=q0['pslot'],
                                       c=q0['c'], d=q0['d'], n=n))
            i = j

    return dict(chunk_sizes=chunk_sizes, prod_instrs=prod_instrs,
                acc_instrs=acc_instrs, needs_memset=needs_memset)


# ----------------------------------------------------------------------------
# Bass program
# ----------------------------------------------------------------------------

def _build_bass(plan, rows_per_core, rep_dim, out_dim, repeat=1, compute_repeat=1):
    import concourse.bass as bass
    import concourse.mybir as mybir
    from concourse.ap import AP
    from concourse.tile import TileContext
    import concourse.tile as _tile_mod
    from concourse.vector_clock import ScopedClock as _ScopedClock

    # The kernel-tail Drain instruction waits on every proc lane with
    # outstanding ticks, but its CTRL ISA struct only has room for a couple
    # of embedded sync-wait commands ("Too many sync wait commands" in
    # walrus codegen otherwise).  Split the global-clock wait across
    # several Drain instructions, two procs each (waits already observed by
    # the SP engine are elided by add_sem_waits).
    if not getattr(_tile_mod.TileContext, '_cg_drain_patched', False):
        _orig_dab = _tile_mod.TileContext._drain_and_barrier

        def _split_drain_and_barrier(self, tick_clock, wait_clock):
            gc = tick_clock.global_clock
            VC = type(gc)
            procs = []
            for p in range(27):
                t = gc.peek_next(p) - 1
                if t > 0:
                    procs.append((p, t))
            for i in range(0, len(procs), 1):
                pc = VC()
                for p, t in procs[i:i + 1]:
                    for _ in range(t):
                        pc.advance(p)
                d = self.nc.sync.drain()
                wait_clock.add_sem_waits(d.ins, _ScopedClock({None: pc}))
            self.nc.all_engine_barrier()
            popped = self.nc._tile_sem_poison_stack.pop()
            assert popped is self._sem_poison
            self.nc.clear_and_free_semaphores(list(self.sems.allocated().values()))
            self.nc.all_engine_barrier()

        _tile_mod.TileContext._drain_and_barrier = _split_drain_and_barrier
        _tile_mod.TileContext._cg_drain_patched = True

    f32 = mybir.dt.float32
    T = T_FOLD
    n_groups = rows_per_core // (P_DIM * T)
    assert rows_per_core == n_groups * P_DIM * T

    nc = bass.Bass("TRN2")
    x1d = nc.declare_dram_parameter("x1", [rows_per_core, rep_dim], f32, isOutput=False)
    x2d = nc.declare_dram_parameter("x2", [rows_per_core, rep_dim], f32, isOutput=False)
    outd = nc.declare_dram_parameter("out", [rows_per_core, out_dim], f32, isOutput=True)

    def ap_custom(tile, base, dims):
        a = tile[:]
        aplist = [list(a.ap[0])] + [[s, n] for (s, n) in dims]
        return AP(a.tensor, a.offset + base, aplist)

    with TileContext(nc) as tc:
        with (
            tc.tile_pool(name="io", bufs=2) as iop,
            tc.tile_pool(name="pp", bufs=2) as ppp,
        ):
            def dram_group_ap(dram, g, width):
                # [128p, T, width] view of rows [g*T*128, (g+1)*T*128):
                # row = g*T*128 + t*128 + p, iterated (p, t, f)
                a = dram[:]
                return AP(a.tensor, g * T * P_DIM * width,
                          [[width, P_DIM], [P_DIM * width, T], [1, width]])

            for g in range(n_groups * repeat):
                g = g % n_groups
                X1 = iop.tile([P_DIM, T * rep_dim], f32, tag="X1")
                X2 = iop.tile([P_DIM, T * rep_dim], f32, tag="X2")
                O = iop.tile([P_DIM, T * out_dim], f32, tag="O")
                # one big DMA per tensor: >=1MiB transfers split across all
                # 16 SDMA engines, and compute instructions then wait on at
                # most a couple of DMA semaphores (HW wait-slot limit).
                nc.gpsimd.dma_start(X1[:], dram_group_ap(x1d, g, rep_dim))
                nc.gpsimd.dma_start(X2[:], dram_group_ap(x2d, g, rep_dim))
                # wait absorbers: 4D-AP TensorTensor instructions cannot
                # carry embedded sync waits (S3S3D3 struct), so soak up the
                # DMA-complete waits with tiny 2D copies first.
                SCR = iop.tile([P_DIM, 4], f32, tag="SCR")
                nc.vector.tensor_copy(SCR[:, 0:2], X1[:, 0:2])
                nc.vector.tensor_copy(SCR[:, 2:4], X2[:, 0:2])
                if plan['needs_memset']:
                    nc.gpsimd.memset(O[:], 0.0)

                for _rep in range(compute_repeat):
                    for ci, csz in enumerate(plan['chunk_sizes']):
                        P = ppp.tile([P_DIM, T * csz], f32, tag="P")
                        for pi in plan['prod_instrs'][ci]:
                            dims = [(csz, T), (pi['ds'], pi['n']), (1, pi['d'])]
                            nc.vector.tensor_tensor(
                                ap_custom(P, pi['pslot'], dims),
                                ap_custom(X1, pi['a'],
                                          [(rep_dim, T), (pi['da'], pi['n']), (1, pi['d'])]),
                                ap_custom(X2, pi['b'],
                                          [(rep_dim, T), (pi['db'], pi['n']), (1, pi['d'])]),
                                mybir.AluOpType.mult,
                            )
                        for qi in plan['acc_instrs'][ci]:
                            w = qi['n'] * qi['d']   # collapsed contiguous width
                            o_ap = ap_custom(O, qi['o'], [(out_dim, T), (1, w)])
                            p_ap = ap_custom(P, qi['pslot'], [(csz, T), (1, w)])
                            if qi['kind'] == 'TS':
                                nc.vector.tensor_scalar_mul(o_ap, p_ap, float(qi['c']))
                            else:
                                nc.vector.scalar_tensor_tensor(
                                    out=o_ap, in0=p_ap, scalar=float(qi['c']),
                                    in1=o_ap,
                                    op0=mybir.AluOpType.mult,
                                    op1=mybir.AluOpType.add,
                                )

                nc.sync.dma_start(dram_group_ap(outd, g, out_dim), O[:])
    return nc


# ----------------------------------------------------------------------------
# Entry point
# ----------------------------------------------------------------------------

def kernel(x1, x2, cg_tilde, repids_in1, repids_in2, repids_out, out_dim):
    from concourse.bass_utils import run_bass_kernel_spmd

    x1 = np.asarray(x1, dtype=np.float32)
    x2 = np.asarray(x2, dtype=np.float32)
    cg = np.asarray(cg_tilde, dtype=np.float32)
    r1 = np.asarray(repids_in1).astype(np.int64)
    r2 = np.asarray(repids_in2).astype(np.int64)
    ro = np.asarray(repids_out).astype(np.int64)
    out_dim = int(out_dim)

    n, rep_dim = x1.shape
    rows_per_core = n // N_CORES

    key = (rows_per_core, rep_dim, out_dim, cg.tobytes(), r1.tobytes(),
           r2.tobytes(), ro.tobytes())
    cache_key = hash(key)
    if cache_key not in _BUILD_CACHE:
        plan = _build_plan(cg, r1, r2, ro, out_dim)
        nc = _build_bass(plan, rows_per_core, rep_dim, out_dim)
        _BUILD_CACHE[cache_key] = nc
    nc = _BUILD_CACHE[cache_key]

    in_maps = [
        {"x1": x1[i*rows_per_core:(i+1)*rows_per_core],
         "x2": x2[i*rows_per_core:(i+1)*rows_per_core]}
        for i in range(N_CORES)
    ]
    res = run_bass_kernel_spmd(nc, in_maps, list(range(N_CORES)))
    out = np.concatenate([res.results[i]["out"] for i in range(N_CORES)], axis=0)
    return out



# revision 2
# speedup vs baseline: 1.2207x; 1.2207x over previous
"""CGCoupler Trainium2 Bass kernel, v2 — PE scatter-accumulate design.

out[n, ro[k]] += x1[n, r1[k]] * x2[n, r2[k]] * cg[k]

The index tables decompose into 147 contiguous-slice FMAs per row
(70 distinct product slices, 19 distinct CG coefficients).  Engine split:

  gpsimd  SWDGE DMA-in with f32->bf16 cast
  DVE     products P = x1_a * x2_b   (bf16 tensor_tensor, 2x mode)
  PE      out[:, o:o+w] += c * P[:, p:p+w]  as matmul(lhsT=c*I_128, rhs=P)
          accumulating in PSUM fp32 (has_written per-bank choreography)
  ACT     PSUM -> SBUF evacuation (fp32)
  SP      HWDGE DMA-out (f32)

Rows on the 128 partitions, T=4 row-tiles folded per instruction.  PSUM
holds the 640 out cols t-innermost ((col*T + t) addressing) split into
three chunks A=[0,256) B=[256,512) C=[512,640) of 2/2/1 banks; every
accum slice is 32/64-wide and 32/64-aligned so no matmul output crosses
a PSUM bank.  Data-parallel across 8 NeuronCores: 2048 rows each.
"""
import numpy as np
import ml_dtypes

N_CORES = 8
P_DIM = 128
T_FOLD = 4
REP_DIM = 640
OUT_DIM = 640
CHUNK_BOUNDS = (0, 256, 512, 640)

_BUILD_CACHE = {}


# ----------------------------------------------------------------------------
# Planning
# ----------------------------------------------------------------------------

def _extract_sliceops(cg, r1, r2, ro):
    M = len(cg)
    ops = []
    k = 0
    while k < M:
        j = k + 1
        while (j < M and r1[j] == r1[j-1] + 1 and r2[j] == r2[j-1] + 1
               and ro[j] == ro[j-1] + 1 and cg[j] == cg[k]):
            j += 1
        ops.append((int(r1[k]), int(r2[k]), int(ro[k]), j - k, float(cg[k])))
        k = j
    return ops


def _build_plan(cg, r1, r2, ro, out_dim, T=T_FOLD):
    ops = _extract_sliceops(cg, r1, r2, ro)
    n_chunks = len(CHUNK_BOUNDS) - 1

    def chunk_of(o):
        for ci in range(n_chunks):
            if o < CHUNK_BOUNDS[ci + 1]:
                return ci
        raise ValueError(o)

    # ---- product pair slots, assigned per earliest-consumer chunk ----
    pair_first_chunk = {}
    for (a, b, o, d, c) in ops:
        key = (a, b, d)
        ci = chunk_of(o)
        if key not in pair_first_chunk or ci < pair_first_chunk[key]:
            pair_first_chunk[key] = ci
    pair_slot = {}          # (a,b,d) -> (chunk, offset)
    chunk_sizes = [0] * n_chunks
    for (a, b, o, d, c) in ops:   # first-use order within chunk
        key = (a, b, d)
        if key in pair_slot:
            continue
        ci = pair_first_chunk[key]
        pair_slot[key] = (ci, chunk_sizes[ci])
        chunk_sizes[ci] += d

    # ---- product instructions per chunk: merge const-stride runs (4D TT) ----
    pair_by_chunk = [[] for _ in range(n_chunks)]
    seen = set()
    for (a, b, o, d, c) in ops:
        key = (a, b, d)
        if key in seen:
            continue
        seen.add(key)
        ci, off = pair_slot[key]
        pair_by_chunk[ci].append((a, b, d, off))
    prod_instrs = [[] for _ in range(n_chunks)]
    for ci in range(n_chunks):
        ch = pair_by_chunk[ci]
        i = 0
        while i < len(ch):
            a0, b0, d0, s0 = ch[i]
            j = i + 1
            da = db = ds = None
            while j < len(ch):
                a1, b1, d1, s1 = ch[j]
                if d1 != d0:
                    break
                nda = a1 - ch[j-1][0]
                ndb = b1 - ch[j-1][1]
                nds = s1 - ch[j-1][3]
                if da is None:
                    da, db, ds = nda, ndb, nds
                elif (nda, ndb, nds) != (da, db, ds):
                    break
                j += 1
            n = j - i
            if n == 1:
                da = db = ds = 0
            prod_instrs[ci].append(dict(pslot=s0, a=a0, b=b0, d=d0,
                                        da=da, db=db, ds=ds, n=n))
            i = j

    # ---- distinct coefficients ----
    cvals = sorted(set(c for (_, _, _, _, c) in ops))
    cidx = {c: i for i, c in enumerate(cvals)}

    # ---- accumulation matmuls per chunk ----
    # The PSUM has_written model (and the simulator's stricter byte-level
    # check) requires each matmul's out range to be either entirely
    # first-touch or entirely already-written.  Per bank we pick a COVER:
    # a set of pairwise-disjoint slices covering all touched columns
    # (w=64 slices are 64-aligned so one covers its whole 64-block; w=32
    # slices cover remaining 32-blocks).  Covers are emitted before all
    # other (fully covered) slices.
    raw = []
    for (a, b, o, d, c) in ops:
        pc, off = pair_slot[(a, b, d)]
        ci = chunk_of(o)
        base = CHUNK_BOUNDS[ci]
        bank = ((o - base) * T * 4) // 2048
        assert bank == (((o - base) + d) * T * 4 - 1) // 2048, (o, d)
        raw.append(dict(o=o, w=d, c=c, pc=pc, pslot=off, ci=ci, bank=bank))

    mms = [[] for _ in range(n_chunks)]
    for ci in range(n_chunks):
        chunk = [q for q in raw if q['ci'] == ci]
        banks = sorted(set(q['bank'] for q in chunk))
        all_covers, all_rest = [], []
        for b in banks:
            sl = [q for q in chunk if q['bank'] == b]
            # cover selection
            cov64 = {}
            for q in sl:
                if q['w'] == 64 and q['o'] not in cov64:
                    cov64[q['o']] = q
            covered = set()
            for q in cov64.values():
                covered.update(range(q['o'], q['o'] + 64, 32))
            cov32 = {}
            for q in sl:
                if (q['w'] == 32 and q['o'] not in covered
                        and q['o'] not in cov32):
                    cov32[q['o']] = q
            cover = set(id(q) for q in cov64.values()) | set(
                id(q) for q in cov32.values())
            covered.update(k for k in cov32)
            for q in sl:   # every touched 32-block must be covered
                for blk in range(q['o'], q['o'] + q['w'], 32):
                    assert blk in covered, (q, covered)
            covers = sorted((q for q in sl if id(q) in cover),
                            key=lambda q: (cidx[q['c']], q['o']))
            rest = sorted((q for q in sl if id(q) not in cover),
                          key=lambda q: (cidx[q['c']], q['o']))
            # merge within each class: same c, contiguous o AND pslot,
            # same P chunk, same bank
            for group, acc in ((covers, all_covers), (rest, all_rest)):
                i = 0
                while i < len(group):
                    q0 = dict(group[i])
                    j = i + 1
                    while (j < len(group)
                           and group[j]['c'] == q0['c']
                           and group[j]['pc'] == q0['pc']
                           and group[j]['o'] == q0['o'] + q0['w']
                           and group[j]['pslot'] == q0['pslot'] + q0['w']):
                        q0['w'] += group[j]['w']
                        j += 1
                    acc.append(q0)
                    i = j
        # all covers before all rest (per-bank first-touch invariant holds),
        # each sorted by coefficient to maximize identical-LDWEIGHTS runs
        ordered = (sorted(all_covers, key=lambda q: (cidx[q['c']], q['bank'], q['o']))
                   + sorted(all_rest, key=lambda q: (cidx[q['c']], q['bank'], q['o'])))
        # start/stop flags per bank in emission order
        first_seen, last_idx = set(), {}
        for i, m in enumerate(ordered):
            m['start'] = m['bank'] not in first_seen
            first_seen.add(m['bank'])
            last_idx[m['bank']] = i
        for i, m in enumerate(ordered):
            m['stop'] = (last_idx[m['bank']] == i)
        mms[ci] = ordered

    return dict(chunk_sizes=chunk_sizes, prod_instrs=prod_instrs,
                mms=mms, cvals=cvals, cidx=cidx)


def _build_weights(cvals):
    n_c = len(cvals)
    w = np.zeros((P_DIM, n_c * P_DIM), dtype=ml_dtypes.bfloat16)
    for i, c in enumerate(cvals):
        blk = w[:, i * P_DIM:(i + 1) * P_DIM]
        np.fill_diagonal(blk, ml_dtypes.bfloat16(c))
    return w


# ----------------------------------------------------------------------------
# Bass program
# ----------------------------------------------------------------------------

def _build_bass(plan, rows_per_core, rep_dim, out_dim, repeat=1,
                compute_repeat=1, dma_only=False, plain_dma=False):
    import concourse.bass as bass
    import concourse.mybir as mybir
    from concourse.ap import AP
    from concourse.tile import TileContext
    import concourse.tile as _tile_mod
    from concourse.vector_clock import ScopedClock as _ScopedClock

    # kernel-tail Drain instruction can only hold a couple of embedded sync
    # waits; split the global-clock wait across several Drains (see baseline).
    if not getattr(_tile_mod.TileContext, '_cg_drain_patched', False):
        def _split_drain_and_barrier(self, tick_clock, wait_clock):
            gc = tick_clock.global_clock
            VC = type(gc)
            procs = []
            for p in range(27):
                t = gc.peek_next(p) - 1
                if t > 0:
                    procs.append((p, t))
            for i in range(0, len(procs), 1):
                pc = VC()
                for p, t in procs[i:i + 1]:
                    for _ in range(t):
                        pc.advance(p)
                d = self.nc.sync.drain()
                wait_clock.add_sem_waits(d.ins, _ScopedClock({None: pc}))
            self.nc.all_engine_barrier()
            popped = self.nc._tile_sem_poison_stack.pop()
            assert popped is self._sem_poison
            self.nc.clear_and_free_semaphores(list(self.sems.allocated().values()))
            self.nc.all_engine_barrier()

        _tile_mod.TileContext._drain_and_barrier = _split_drain_and_barrier
        _tile_mod.TileContext._cg_drain_patched = True

    f32 = mybir.dt.float32
    bf16 = mybir.dt.bfloat16
    T = T_FOLD
    n_groups = rows_per_core // (P_DIM * T)
    assert rows_per_core == n_groups * P_DIM * T
    n_chunks = len(CHUNK_BOUNDS) - 1
    n_c = len(plan['cvals'])
    csz = plan['chunk_sizes']

    nc = bass.Bass("TRN2")
    x1d = nc.declare_dram_parameter("x1", [rows_per_core, rep_dim], f32, isOutput=False)
    x2d = nc.declare_dram_parameter("x2", [rows_per_core, rep_dim], f32, isOutput=False)
    cgw = nc.declare_dram_parameter("cgw", [P_DIM, n_c * P_DIM], bf16, isOutput=False)
    outd = nc.declare_dram_parameter("out", [rows_per_core, out_dim], f32, isOutput=True)

    def ap_custom(tile, base, dims):
        a = tile[:]
        aplist = [list(a.ap[0])] + [[s, n] for (s, n) in dims]
        return AP(a.tensor, a.offset + base, aplist)

    def dram_group_ap(dram, g, width):
        a = dram[:]
        return AP(a.tensor, g * T * P_DIM * width,
                  [[width, P_DIM], [P_DIM * width, T], [1, width]])

    with TileContext(nc) as tc:
        with (
            tc.tile_pool(name="const", bufs=1) as cstp,
            tc.tile_pool(name="io", bufs=n_groups) as iop,
            tc.tile_pool(name="ob", bufs=n_groups) as obp,
            tc.tile_pool(name="pp", bufs=2) as ppp,
            tc.tile_pool(name="ps", bufs=1, space="PSUM") as psp,
        ):
            W = cstp.tile([P_DIM, n_c * P_DIM], bf16, tag="W")
            nc.sync.dma_start(W[:], cgw[:])
            # dep-free initialized source for absorber copies (Pool-written
            # once; DVE readers carry at most that one Pool wait)
            JUNK = cstp.tile([P_DIM, 4], bf16, tag="JUNK")
            nc.gpsimd.memset(JUNK[:], 0.0)
            if dma_only:
                # out DRAM must be written once (bf16->f32 cast SWDGE dump)
                nc.gpsimd.dma_start(
                    AP(outd[:].tensor, 0, [[out_dim, 1], [1, 4]]),
                    JUNK[0:1, 0:4])
            # Persistent PSUM tiles are created below; the W-DMA wait is
            # absorbed by a warmup dummy matmul right after them.

            # Persistent PSUM tiles (acquired once; cross-group hazards are
            # handled by the byte-level tracker + dummy-matmul absorbers).
            PS = [psp.tile([P_DIM, T * (CHUNK_BOUNDS[ci+1] - CHUNK_BOUNDS[ci])],
                           f32, tag=f"PS{ci}", name=f"PS{ci}")
                  for ci in range(n_chunks)]
            if not dma_only:
                # PE warmup absorber: takes the weights-DMA wait so the first
                # real matmul carries only its DVE wait.
                nc.tensor.matmul(
                    ap_custom(PS[0], 0, [(1, 2)]),
                    ap_custom(W, 0, [(1, P_DIM)]),
                    W[:, 0:2], start=True, stop=True)

            # Wait-slot budget choreography: several instruction structs
            # (cast-DMA, 4D TensorTensor) carry at most one / zero embedded
            # sync waits, so every cross-engine dependency is "absorbed" by
            # a tiny wait-capable instruction on the consuming engine first.
            last_mm = plan['mms'][-1][-1]          # last-emitted matmul
            O_prev = None
            for gi in range(n_groups * repeat):
                g = gi % n_groups
                xdt = f32 if plain_dma else bf16
                X1 = iop.tile([P_DIM, T * rep_dim], xdt, tag="X1")
                X2 = iop.tile([P_DIM, T * rep_dim], xdt, tag="X2")
                O = None if dma_only else obp.tile([P_DIM, T * out_dim],
                                                   f32, tag="O")
                # SWDGE DMA with f32 -> bf16 cast in flight
                nc.gpsimd.dma_start(X1[:], dram_group_ap(x1d, g, rep_dim))
                nc.gpsimd.dma_start(X2[:], dram_group_ap(x2d, g, rep_dim))
                # DVE absorbers take the DMA-complete waits so the 4D
                # products carry none.
                if not dma_only:
                    SCR = obp.tile([P_DIM, 8], bf16, tag="SCR")
                    nc.vector.tensor_copy(SCR[:, 0:2], X1[:, 0:2])
                    nc.vector.tensor_copy(SCR[:, 2:4], X2[:, 0:2])

                P_prev = None
                for _rep in range(0 if dma_only else compute_repeat):
                    P = [ppp.tile([P_DIM, T * csz[ci]], bf16, tag=f"P{ci}",
                                  name=f"P{ci}")
                         for ci in range(n_chunks)]
                    if _rep > 0:
                        # DVE self-observer: RAW read of the previous rep's
                        # last product output raises DVE's observed self-sem
                        # value so cross-rep WAW waits below are elided.
                        lpc = plan['prod_instrs'][-1][-1]
                        nc.vector.tensor_copy(
                            SCR[:, 6:8],
                            ap_custom(P_prev[n_chunks - 1], lpc['pslot'],
                                      [(1, 2)]))
                    # DVE absorber for the P-buffer WAR vs last iteration's
                    # matmuls: write the byte range read by the last-emitted
                    # matmul (products overwrite it with real data after).
                    nc.vector.tensor_copy(
                        ap_custom(P[last_mm['pc']], last_mm['pslot'], [(1, 2)]),
                        JUNK[:, 0:2])
                    last_rep = (_rep == compute_repeat - 1)
                    # scheduler-only fence: the absorber copies above must be
                    # scheduled before the products/matmuls below for their
                    # wait-elision to apply (no semaphores synthesized)
                    tc.no_sync_barrier()

                    # phase 1: all products (DVE, bf16 2x)
                    for ci in range(n_chunks):
                        for pi in plan['prod_instrs'][ci]:
                            dims = [(csz[ci], T), (pi['ds'], pi['n']), (1, pi['d'])]
                            nc.vector.tensor_tensor(
                                ap_custom(P[ci], pi['pslot'], dims),
                                ap_custom(X1, pi['a'],
                                          [(rep_dim, T), (pi['da'], pi['n']), (1, pi['d'])]),
                                ap_custom(X2, pi['b'],
                                          [(rep_dim, T), (pi['db'], pi['n']), (1, pi['d'])]),
                                mybir.AluOpType.mult,
                            )
                    # phase 2: scatter-accumulate (PE)
                    for ci in range(n_chunks):
                        base = CHUNK_BOUNDS[ci]
                        # PE-self-wait absorber: a 2-element self-contained
                        # dummy matmul overlapping the previous iteration's
                        # last-emitted matmul output takes the PSUM
                        # write-drain WAW wait, so the real matmuls below
                        # carry at most one (DVE) wait.
                        lm = plan['mms'][ci][-1]
                        nc.tensor.matmul(
                            ap_custom(PS[ci], (lm['o'] - base) * T, [(1, 2)]),
                            ap_custom(W, 0, [(1, P_DIM)]),
                            JUNK[:, 0:2], start=True, stop=True)
                        for m in plan['mms'][ci]:
                            wi = plan['cidx'][m['c']]
                            lhsT = ap_custom(W, wi * P_DIM, [(1, P_DIM)])
                            rhs = ap_custom(P[m['pc']], m['pslot'],
                                            [(1, m['w']), (csz[m['pc']], T)])
                            out_ap = ap_custom(PS[ci], (m['o'] - base) * T,
                                               [(1, m['w'] * T)])
                            nc.tensor.matmul(out_ap, lhsT, rhs,
                                             start=m['start'], stop=m['stop'])
                    # phase 3 (last rep only): evacuations on the otherwise
                    # idle ACT engine so they never stall the DVE product
                    # stream.  The ACT-side absorber (RAW read of the
                    # previous group's last evacuation output) raises ACT's
                    # observed self-sem value so the cross-group PSUM
                    # read-read waits on the evacuations are elided.
                    if last_rep:
                        if O_prev is not None:
                            SCRO = obp.tile([P_DIM, 2], f32, tag="SCRO")
                            nc.scalar.copy(SCRO[:, 0:2],
                                           O_prev[:, T * out_dim - 2:])
                        O_prev = O
                        for ci in range(n_chunks):
                            base = CHUNK_BOUNDS[ci]
                            cw = CHUNK_BOUNDS[ci+1] - base
                            nc.scalar.copy(
                                ap_custom(O, base, [(out_dim, T), (1, cw)]),
                                ap_custom(PS[ci], 0, [(1, T), (T, cw)]),
                            )
                    P_prev = P

                if not dma_only:
                    nc.sync.dma_start(dram_group_ap(outd, g, out_dim), O[:])
    return nc


# ----------------------------------------------------------------------------
# Entry point
# ----------------------------------------------------------------------------

def kernel(x1, x2, cg_tilde, repids_in1, repids_in2, repids_out, out_dim):
    from concourse.bass_utils import run_bass_kernel_spmd

    x1 = np.asarray(x1, dtype=np.float32)
    x2 = np.asarray(x2, dtype=np.float32)
    cg = np.asarray(cg_tilde, dtype=np.float32)
    r1 = np.asarray(repids_in1).astype(np.int64)
    r2 = np.asarray(repids_in2).astype(np.int64)
    ro = np.asarray(repids_out).astype(np.int64)
    out_dim = int(out_dim)

    n, rep_dim = x1.shape
    rows_per_core = n // N_CORES

    key = (rows_per_core, rep_dim, out_dim, cg.tobytes(), r1.tobytes(),
           r2.tobytes(), ro.tobytes())
    cache_key = hash(key)
    if cache_key not in _BUILD_CACHE:
        plan = _build_plan(cg, r1, r2, ro, out_dim)
        nc = _build_bass(plan, rows_per_core, rep_dim, out_dim)
        _BUILD_CACHE[cache_key] = (nc, _build_weights(plan['cvals']))
    nc, wmat = _BUILD_CACHE[cache_key]

    in_maps = [
        {"x1": x1[i*rows_per_core:(i+1)*rows_per_core],
         "x2": x2[i*rows_per_core:(i+1)*rows_per_core],
         "cgw": wmat}
        for i in range(N_CORES)
    ]
    res = run_bass_kernel_spmd(nc, in_maps, list(range(N_CORES)))
    out = np.concatenate([res.results[i]["out"] for i in range(N_CORES)], axis=0)
    return out


# revision 3
# speedup vs baseline: 2.0280x; 1.6614x over previous
"""CGCoupler Trainium2 Bass kernel, v2 — PE scatter-accumulate design.

out[n, ro[k]] += x1[n, r1[k]] * x2[n, r2[k]] * cg[k]

The index tables decompose into 147 contiguous-slice FMAs per row
(70 distinct product slices, 19 distinct CG coefficients).  Engine split:

  gpsimd  SWDGE DMA-in with f32->bf16 cast
  DVE     products P = x1_a * x2_b   (bf16 tensor_tensor, 2x mode)
  PE      out[:, o:o+w] += c * P[:, p:p+w]  as matmul(lhsT=c*I_128, rhs=P)
          accumulating in PSUM fp32 (has_written per-bank choreography)
  ACT     PSUM -> SBUF evacuation (fp32)
  SP      HWDGE DMA-out (f32)

Rows on the 128 partitions, T=4 row-tiles folded per instruction.  PSUM
holds the 640 out cols t-innermost ((col*T + t) addressing) split into
three chunks A=[0,256) B=[256,512) C=[512,640) of 2/2/1 banks; every
accum slice is 32/64-wide and 32/64-aligned so no matmul output crosses
a PSUM bank.  Data-parallel across 8 NeuronCores: 2048 rows each.
"""
import numpy as np
import ml_dtypes

N_CORES = 8
P_DIM = 128
T_FOLD = 4
REP_DIM = 640
OUT_DIM = 640
CHUNK_BOUNDS = (0, 256, 512, 640)

_BUILD_CACHE = {}


# ----------------------------------------------------------------------------
# Planning
# ----------------------------------------------------------------------------

def _extract_sliceops(cg, r1, r2, ro):
    M = len(cg)
    ops = []
    k = 0
    while k < M:
        j = k + 1
        while (j < M and r1[j] == r1[j-1] + 1 and r2[j] == r2[j-1] + 1
               and ro[j] == ro[j-1] + 1 and cg[j] == cg[k]):
            j += 1
        ops.append((int(r1[k]), int(r2[k]), int(ro[k]), j - k, float(cg[k])))
        k = j
    return ops


def _build_plan(cg, r1, r2, ro, out_dim, T=T_FOLD):
    ops = _extract_sliceops(cg, r1, r2, ro)
    n_chunks = len(CHUNK_BOUNDS) - 1

    def chunk_of(o):
        for ci in range(n_chunks):
            if o < CHUNK_BOUNDS[ci + 1]:
                return ci
        raise ValueError(o)

    # ---- product pair slots, assigned per earliest-consumer chunk ----
    pair_first_chunk = {}
    for (a, b, o, d, c) in ops:
        key = (a, b, d)
        ci = chunk_of(o)
        if key not in pair_first_chunk or ci < pair_first_chunk[key]:
            pair_first_chunk[key] = ci
    pair_slot = {}          # (a,b,d) -> (chunk, offset)
    chunk_sizes = [0] * n_chunks
    for (a, b, o, d, c) in ops:   # first-use order within chunk
        key = (a, b, d)
        if key in pair_slot:
            continue
        ci = pair_first_chunk[key]
        pair_slot[key] = (ci, chunk_sizes[ci])
        chunk_sizes[ci] += d

    # ---- product instructions per chunk: merge const-stride runs (4D TT) ----
    pair_by_chunk = [[] for _ in range(n_chunks)]
    seen = set()
    for (a, b, o, d, c) in ops:
        key = (a, b, d)
        if key in seen:
            continue
        seen.add(key)
        ci, off = pair_slot[key]
        pair_by_chunk[ci].append((a, b, d, off))
    prod_instrs = [[] for _ in range(n_chunks)]
    for ci in range(n_chunks):
        ch = pair_by_chunk[ci]
        i = 0
        while i < len(ch):
            a0, b0, d0, s0 = ch[i]
            j = i + 1
            da = db = ds = None
            while j < len(ch):
                a1, b1, d1, s1 = ch[j]
                if d1 != d0:
                    break
                nda = a1 - ch[j-1][0]
                ndb = b1 - ch[j-1][1]
                nds = s1 - ch[j-1][3]
                if da is None:
                    da, db, ds = nda, ndb, nds
                elif (nda, ndb, nds) != (da, db, ds):
                    break
                j += 1
            n = j - i
            if n == 1:
                da = db = ds = 0
            prod_instrs[ci].append(dict(pslot=s0, a=a0, b=b0, d=d0,
                                        da=da, db=db, ds=ds, n=n))
            i = j

    # ---- distinct coefficients ----
    cvals = sorted(set(c for (_, _, _, _, c) in ops))
    cidx = {c: i for i, c in enumerate(cvals)}

    # ---- accumulation matmuls per chunk ----
    # The PSUM has_written model (and the simulator's stricter byte-level
    # check) requires each matmul's out range to be either entirely
    # first-touch or entirely already-written.  Per bank we pick a COVER:
    # a set of pairwise-disjoint slices covering all touched columns
    # (w=64 slices are 64-aligned so one covers its whole 64-block; w=32
    # slices cover remaining 32-blocks).  Covers are emitted before all
    # other (fully covered) slices.
    raw = []
    for (a, b, o, d, c) in ops:
        pc, off = pair_slot[(a, b, d)]
        ci = chunk_of(o)
        base = CHUNK_BOUNDS[ci]
        bank = ((o - base) * T * 4) // 2048
        assert bank == (((o - base) + d) * T * 4 - 1) // 2048, (o, d)
        raw.append(dict(o=o, w=d, c=c, pc=pc, pslot=off, ci=ci, bank=bank))

    mms = [[] for _ in range(n_chunks)]
    for ci in range(n_chunks):
        chunk = [q for q in raw if q['ci'] == ci]
        banks = sorted(set(q['bank'] for q in chunk))
        all_covers, all_rest = [], []
        for b in banks:
            sl = [q for q in chunk if q['bank'] == b]
            # cover selection
            cov64 = {}
            for q in sl:
                if q['w'] == 64 and q['o'] not in cov64:
                    cov64[q['o']] = q
            covered = set()
            for q in cov64.values():
                covered.update(range(q['o'], q['o'] + 64, 32))
            cov32 = {}
            for q in sl:
                if (q['w'] == 32 and q['o'] not in covered
                        and q['o'] not in cov32):
                    cov32[q['o']] = q
            cover = set(id(q) for q in cov64.values()) | set(
                id(q) for q in cov32.values())
            covered.update(k for k in cov32)
            for q in sl:   # every touched 32-block must be covered
                for blk in range(q['o'], q['o'] + q['w'], 32):
                    assert blk in covered, (q, covered)
            covers = sorted((q for q in sl if id(q) in cover),
                            key=lambda q: (cidx[q['c']], q['o']))
            rest = sorted((q for q in sl if id(q) not in cover),
                          key=lambda q: (cidx[q['c']], q['o']))
            # merge within each class: same c, contiguous o AND pslot,
            # same P chunk, same bank
            for group, acc in ((covers, all_covers), (rest, all_rest)):
                i = 0
                while i < len(group):
                    q0 = dict(group[i])
                    j = i + 1
                    while (j < len(group)
                           and group[j]['c'] == q0['c']
                           and group[j]['pc'] == q0['pc']
                           and group[j]['o'] == q0['o'] + q0['w']
                           and group[j]['pslot'] == q0['pslot'] + q0['w']):
                        q0['w'] += group[j]['w']
                        j += 1
                    acc.append(q0)
                    i = j
        if ci == n_chunks - 1:
            # chunk C runs on DVE (TS first-touch + STT accumulate): the
            # per-matmul overhead on PE dominates for these 26 small slices
            mms[ci] = dict(covers=all_covers, rest=all_rest)
            continue
        # all covers before all rest (per-bank first-touch invariant holds),
        # each sorted by coefficient to maximize identical-LDWEIGHTS runs
        ordered = (sorted(all_covers, key=lambda q: (cidx[q['c']], q['bank'], q['o']))
                   + sorted(all_rest, key=lambda q: (cidx[q['c']], q['bank'], q['o'])))
        # start/stop flags per bank in emission order
        first_seen, last_idx = set(), {}
        for i, m in enumerate(ordered):
            m['start'] = m['bank'] not in first_seen
            first_seen.add(m['bank'])
            last_idx[m['bank']] = i
        for i, m in enumerate(ordered):
            m['stop'] = (last_idx[m['bank']] == i)
        mms[ci] = ordered

    return dict(chunk_sizes=chunk_sizes, prod_instrs=prod_instrs,
                mms=mms, cvals=cvals, cidx=cidx)


def _build_weights(cvals):
    n_c = len(cvals)
    w = np.zeros((P_DIM, n_c * P_DIM), dtype=ml_dtypes.bfloat16)
    for i, c in enumerate(cvals):
        blk = w[:, i * P_DIM:(i + 1) * P_DIM]
        np.fill_diagonal(blk, ml_dtypes.bfloat16(c))
    return w


# ----------------------------------------------------------------------------
# Bass program
# ----------------------------------------------------------------------------

def _build_bass(plan, rows_per_core, rep_dim, out_dim, repeat=1,
                compute_repeat=1, dma_only=False, plain_dma=False):
    import concourse.bass as bass
    import concourse.mybir as mybir
    from concourse.ap import AP
    from concourse.tile import TileContext
    import concourse.tile as _tile_mod
    from concourse.vector_clock import ScopedClock as _ScopedClock

    # kernel-tail Drain instruction can only hold a couple of embedded sync
    # waits; split the global-clock wait across several Drains (see baseline).
    if not getattr(_tile_mod.TileContext, '_cg_drain_patched', False):
        def _split_drain_and_barrier(self, tick_clock, wait_clock):
            gc = tick_clock.global_clock
            VC = type(gc)
            procs = []
            for p in range(27):
                t = gc.peek_next(p) - 1
                if t > 0:
                    procs.append((p, t))
            for i in range(0, len(procs), 1):
                pc = VC()
                for p, t in procs[i:i + 1]:
                    for _ in range(t):
                        pc.advance(p)
                d = self.nc.sync.drain()
                wait_clock.add_sem_waits(d.ins, _ScopedClock({None: pc}))
            self.nc.all_engine_barrier()
            popped = self.nc._tile_sem_poison_stack.pop()
            assert popped is self._sem_poison
            self.nc.clear_and_free_semaphores(list(self.sems.allocated().values()))
            self.nc.all_engine_barrier()

        _tile_mod.TileContext._drain_and_barrier = _split_drain_and_barrier
        _tile_mod.TileContext._cg_drain_patched = True

    f32 = mybir.dt.float32
    bf16 = mybir.dt.bfloat16
    T = T_FOLD
    n_groups = rows_per_core // (P_DIM * T)
    assert rows_per_core == n_groups * P_DIM * T
    n_chunks = len(CHUNK_BOUNDS) - 1
    n_c = len(plan['cvals'])
    csz = plan['chunk_sizes']

    nc = bass.Bass("TRN2")
    x1d = nc.declare_dram_parameter("x1", [rows_per_core, rep_dim], f32, isOutput=False)
    x2d = nc.declare_dram_parameter("x2", [rows_per_core, rep_dim], f32, isOutput=False)
    cgw = nc.declare_dram_parameter("cgw", [P_DIM, n_c * P_DIM], bf16, isOutput=False)
    outd = nc.declare_dram_parameter("out", [rows_per_core, out_dim], f32, isOutput=True)

    def ap_custom(tile, base, dims):
        a = tile[:]
        aplist = [list(a.ap[0])] + [[s, n] for (s, n) in dims]
        return AP(a.tensor, a.offset + base, aplist)

    def dram_group_ap(dram, g, width):
        a = dram[:]
        return AP(a.tensor, g * T * P_DIM * width,
                  [[width, P_DIM], [P_DIM * width, T], [1, width]])

    with TileContext(nc) as tc:
        with (
            tc.tile_pool(name="const", bufs=1) as cstp,
            tc.tile_pool(name="io", bufs=n_groups) as iop,
            tc.tile_pool(name="ob", bufs=n_groups) as obp,
            tc.tile_pool(name="pp", bufs=2) as ppp,
            tc.tile_pool(name="ps", bufs=1, space="PSUM") as psp,
        ):
            W = cstp.tile([P_DIM, n_c * P_DIM], bf16, tag="W")
            nc.sync.dma_start(W[:], cgw[:])
            # dep-free initialized source for absorber copies (Pool-written
            # once; DVE readers carry at most that one Pool wait)
            JUNK = cstp.tile([P_DIM, 4], bf16, tag="JUNK")
            nc.gpsimd.memset(JUNK[:], 0.0)
            if dma_only:
                # out DRAM must be written once (bf16->f32 cast SWDGE dump)
                nc.gpsimd.dma_start(
                    AP(outd[:].tensor, 0, [[out_dim, 1], [1, 4]]),
                    JUNK[0:1, 0:4])
            # Persistent PSUM tiles are created below; the W-DMA wait is
            # absorbed by a warmup dummy matmul right after them.

            # Persistent PSUM tiles (acquired once; cross-group hazards are
            # handled by the byte-level tracker + dummy-matmul absorbers).
            PS = [psp.tile([P_DIM, T * (CHUNK_BOUNDS[ci+1] - CHUNK_BOUNDS[ci])],
                           f32, tag=f"PS{ci}", name=f"PS{ci}")
                  for ci in range(n_chunks - 1)]
            if not dma_only:
                # PE warmup absorber: takes the weights-DMA wait so the first
                # real matmul carries only its DVE wait.
                nc.tensor.matmul(
                    ap_custom(PS[0], 0, [(1, 2)]),
                    ap_custom(W, 0, [(1, P_DIM)]),
                    W[:, 0:2], start=True, stop=True)

            # Wait-slot budget choreography: several instruction structs
            # (cast-DMA, 4D TensorTensor) carry at most one / zero embedded
            # sync waits, so every cross-engine dependency is "absorbed" by
            # a tiny wait-capable instruction on the consuming engine first.
            last_mm = plan['mms'][1][-1]           # last-emitted PE matmul
            O_prev = None
            for gi in range(n_groups * repeat):
                g = gi % n_groups
                xdt = f32 if plain_dma else bf16
                X1 = iop.tile([P_DIM, T * rep_dim], xdt, tag="X1")
                X2 = iop.tile([P_DIM, T * rep_dim], xdt, tag="X2")
                ow = CHUNK_BOUNDS[n_chunks - 1]
                cww = out_dim - ow
                O = None if dma_only else obp.tile([P_DIM, T * out_dim],
                                                   f32, tag="O")
                OC = None if dma_only else obp.tile([P_DIM, T * cww],
                                                    bf16, tag="OC")
                # SWDGE DMA with f32 -> bf16 cast in flight
                nc.gpsimd.dma_start(X1[:], dram_group_ap(x1d, g, rep_dim))
                nc.gpsimd.dma_start(X2[:], dram_group_ap(x2d, g, rep_dim))
                # DVE absorbers take the DMA-complete waits so the 4D
                # products carry none.
                if not dma_only:
                    SCR = obp.tile([P_DIM, 8], bf16, tag="SCR")
                    nc.vector.tensor_copy(SCR[:, 0:2], X1[:, 0:2])
                    nc.vector.tensor_copy(SCR[:, 2:4], X2[:, 0:2])

                P_prev = None
                for _rep in range(0 if dma_only else compute_repeat):
                    P = [ppp.tile([P_DIM, T * csz[ci]], bf16, tag=f"P{ci}",
                                  name=f"P{ci}")
                         for ci in range(n_chunks)]
                    if _rep > 0:
                        # DVE self-observer: RAW read of the previous rep's
                        # last product output raises DVE's observed self-sem
                        # value so cross-rep WAW waits below are elided.
                        lpc = plan['prod_instrs'][-1][-1]
                        nc.vector.tensor_copy(
                            SCR[:, 6:8],
                            ap_custom(P_prev[n_chunks - 1], lpc['pslot'],
                                      [(1, 2)]))
                    # DVE absorber for the P-buffer WAR vs last iteration's
                    # matmuls: write the byte range read by the last-emitted
                    # matmul (products overwrite it with real data after).
                    nc.vector.tensor_copy(
                        ap_custom(P[last_mm['pc']], last_mm['pslot'], [(1, 2)]),
                        JUNK[:, 0:2])
                    last_rep = (_rep == compute_repeat - 1)
                    # scheduler-only fence: the absorber copies above must be
                    # scheduled before the products/matmuls below for their
                    # wait-elision to apply (no semaphores synthesized)
                    tc.no_sync_barrier()

                    # phase 1: all products (DVE, bf16 2x)
                    for ci in range(n_chunks):
                        for pi in plan['prod_instrs'][ci]:
                            dims = [(csz[ci], T), (pi['ds'], pi['n']), (1, pi['d'])]
                            nc.vector.tensor_tensor(
                                ap_custom(P[ci], pi['pslot'], dims),
                                ap_custom(X1, pi['a'],
                                          [(rep_dim, T), (pi['da'], pi['n']), (1, pi['d'])]),
                                ap_custom(X2, pi['b'],
                                          [(rep_dim, T), (pi['db'], pi['n']), (1, pi['d'])]),
                                mybir.AluOpType.mult,
                            )
                    # phase 1.5: chunk C on DVE (TS covers + STT accums,
                    # bf16 2x/4x modes) into the OC tile
                    cbase = CHUNK_BOUNDS[n_chunks - 1]
                    cm = plan['mms'][n_chunks - 1]
                    for q in cm['covers']:
                        nc.vector.tensor_scalar_mul(
                            ap_custom(OC, (q['o'] - cbase), [(cww, T), (1, q['w'])]),
                            ap_custom(P[q['pc']], q['pslot'],
                                      [(csz[q['pc']], T), (1, q['w'])]),
                            float(q['c']))
                    for q in cm['rest']:
                        o_ap = ap_custom(OC, (q['o'] - cbase),
                                         [(cww, T), (1, q['w'])])
                        nc.vector.scalar_tensor_tensor(
                            out=o_ap,
                            in0=ap_custom(P[q['pc']], q['pslot'],
                                          [(csz[q['pc']], T), (1, q['w'])]),
                            scalar=float(q['c']), in1=o_ap,
                            op0=mybir.AluOpType.mult,
                            op1=mybir.AluOpType.add)
                    # phase 2: scatter-accumulate (PE)
                    for ci in range(n_chunks - 1):
                        base = CHUNK_BOUNDS[ci]
                        # PE-self-wait absorber: a 2-element self-contained
                        # dummy matmul overlapping the previous iteration's
                        # last-emitted matmul output takes the PSUM
                        # write-drain WAW wait, so the real matmuls below
                        # carry at most one (DVE) wait.
                        lm = plan['mms'][ci][-1]
                        nc.tensor.matmul(
                            ap_custom(PS[ci], (lm['o'] - base) * T, [(1, 2)]),
                            ap_custom(W, 0, [(1, P_DIM)]),
                            JUNK[:, 0:2], start=True, stop=True)
                        for m in plan['mms'][ci]:
                            wi = plan['cidx'][m['c']]
                            lhsT = ap_custom(W, wi * P_DIM, [(1, P_DIM)])
                            rhs = ap_custom(P[m['pc']], m['pslot'],
                                            [(1, m['w']), (csz[m['pc']], T)])
                            out_ap = ap_custom(PS[ci], (m['o'] - base) * T,
                                               [(1, m['w'] * T)])
                            nc.tensor.matmul(out_ap, lhsT, rhs,
                                             start=m['start'], stop=m['stop'])
                    # phase 3 (last rep only): evacuations on the otherwise
                    # idle ACT engine so they never stall the DVE product
                    # stream.  The ACT-side absorber (RAW read of the
                    # previous group's last evacuation output) raises ACT's
                    # observed self-sem value so the cross-group PSUM
                    # read-read waits on the evacuations are elided.
                    if last_rep:
                        if O_prev is not None:
                            SCRO = obp.tile([P_DIM, 2], f32, tag="SCRO")
                            nc.scalar.copy(SCRO[:, 0:2],
                                           O_prev[:, T * out_dim - 2:])
                        O_prev = O
                        for ci in range(n_chunks - 1):
                            base = CHUNK_BOUNDS[ci]
                            cw = CHUNK_BOUNDS[ci+1] - base
                            nc.scalar.copy(
                                ap_custom(O, base, [(out_dim, T), (1, cw)]),
                                ap_custom(PS[ci], 0, [(1, T), (T, cw)]),
                            )
                    P_prev = P

                if not dma_only:
                    # ACT merges the DVE-computed C block into O (bf16->f32
                    # cast), keeping ACT the sole O writer so the single
                    # HWDGE out-DMA carries one wait.
                    nc.scalar.copy(
                        ap_custom(O, ow, [(out_dim, T), (1, cww)]),
                        ap_custom(OC, 0, [(cww, T), (1, cww)]))
                    nc.sync.dma_start(dram_group_ap(outd, g, out_dim), O[:])
    return nc


# ----------------------------------------------------------------------------
# Entry point
# ----------------------------------------------------------------------------

def kernel(x1, x2, cg_tilde, repids_in1, repids_in2, repids_out, out_dim):
    from concourse.bass_utils import run_bass_kernel_spmd

    x1 = np.asarray(x1, dtype=np.float32)
    x2 = np.asarray(x2, dtype=np.float32)
    cg = np.asarray(cg_tilde, dtype=np.float32)
    r1 = np.asarray(repids_in1).astype(np.int64)
    r2 = np.asarray(repids_in2).astype(np.int64)
    ro = np.asarray(repids_out).astype(np.int64)
    out_dim = int(out_dim)

    n, rep_dim = x1.shape
    rows_per_core = n // N_CORES

    key = (rows_per_core, rep_dim, out_dim, cg.tobytes(), r1.tobytes(),
           r2.tobytes(), ro.tobytes())
    cache_key = hash(key)
    if cache_key not in _BUILD_CACHE:
        plan = _build_plan(cg, r1, r2, ro, out_dim)
        nc = _build_bass(plan, rows_per_core, rep_dim, out_dim)
        _BUILD_CACHE[cache_key] = (nc, _build_weights(plan['cvals']))
    nc, wmat = _BUILD_CACHE[cache_key]

    in_maps = [
        {"x1": x1[i*rows_per_core:(i+1)*rows_per_core],
         "x2": x2[i*rows_per_core:(i+1)*rows_per_core],
         "cgw": wmat}
        for i in range(N_CORES)
    ]
    res = run_bass_kernel_spmd(nc, in_maps, list(range(N_CORES)))
    out = np.concatenate([res.results[i]["out"] for i in range(N_CORES)], axis=0)
    return out
